# revision 15
# baseline (speedup 1.0000x reference)
"""NLBlockND multi-cross attention block on 8 Trainium2 NeuronCores.

Per-core shard: core c handles batch b = c//2, spatial half h = c%2
(i in [h*2048, (h+1)*2048)).  Mixed 16-bit datapath: the score path (x, weights,
theta, phi) runs fp16 (10-bit mantissa, scores need the accuracy); the
exp/e path (e, gzT, softmax tree, ones) runs bf16 (e spans ~90 e-folds,
needs the exponent range).  Full PE rate at any moving width + FWL fast
weight loads for both; PSUM/stats fp32.  Softmax uses a constant shift of -50 which cancels in the ratio;
the theta conv bias is constant per softmax row so it cancels exactly
and is dropped; phi's bias is folded into the PSUM drain.  b_g/b_z drop
out in training-mode BN; w_z is folded into w_g on the host.  Softmax
j-reduction: 24 j-blocks tree-reduced on DVE (packed fp16, 2x mode),
8 on GPSIMD.  BN rstd = exp(-0.5*ln(var+eps)) so every activation stays
in one ACT table set (no reloads).  Cross-rep tiles (theta/phi/gzT/
z_sb/stat) are double-buffered so one rep's BN-stats AllGather + apply
tail overlaps the next rep's compute.  BN batch stats are all-gathered
([128,2]) across the 8 cores.
"""
import sys
sys.path.insert(0, '/opt/trn_rl_repo')

import numpy as np

B, CIN, CI, H, W = 4, 256, 128, 64, 64
NJ = H * W              # 4096 (full spatial, j axis)
NI = NJ // 2            # 2048 per-core i positions
IC = 256                # i-chunk
NCH = NI // IC          # 8 chunks
JBS = NJ // 128         # 32 j-blocks
JGROUPS = [4] * 8       # j-blocks per exp group (PSUM: 2 banks)
MD = 24                 # j-blocks tree-reduced on DVE; JBS-MD on GPSIMD
SHIFT = -50.0
BN_EPS = 1e-5
NTOT = float(B * NJ)    # BN count per channel

_CACHE = {}


def _build(unroll=1):
    import concourse.bacc as bacc
    import concourse.mybir as mybir
    from concourse import tile

    dt = mybir.dt
    AF = mybir.ActivationFunctionType
    ALU = mybir.AluOpType

    nc = bacc.Bacc("TRN2", target_bir_lowering=False, debug=False, num_devices=8)

    xtb = nc.dram_tensor("xtb", [CIN, NJ], dt.float16, kind="ExternalInput").ap()
    xob = nc.dram_tensor("xob", [CIN, NI], dt.float16, kind="ExternalInput").ap()
    wtT_d = nc.dram_tensor("wtT", [CIN, CI], dt.float16, kind="ExternalInput").ap()
    wpT_d = nc.dram_tensor("wpT", [CIN, CI], dt.float16, kind="ExternalInput").ap()
    wzgT_d = nc.dram_tensor("wzgT", [CIN, CI], dt.float16, kind="ExternalInput").ap()
    # consts columns: 0 b_phi, 1 gamma, 2 beta, 3 SHIFT, 4 eps, 5 1/NTOT
    consts = nc.dram_tensor("consts", [CI, 8], dt.float32, kind="ExternalInput").ap()
    zout_d = nc.dram_tensor("z", [CI, NI], dt.float32, kind="ExternalOutput").ap()

    cc_in = [nc.dram_tensor(f"cc_in{i}", [CI, 2], dt.float32) for i in range(2)]
    cc_out = [nc.dram_tensor(f"cc_out{i}", [8 * CI, 2], dt.float32,
                             addr_space="Shared") for i in range(2)]

    with tile.TileContext(nc) as tc, \
         nc.allow_low_precision(reason="fp16 datapath, fp32 accumulation"):
        with tc.tile_pool(name="ld", bufs=1) as ldp, \
             tc.tile_pool(name="big", bufs=1) as bigp, \
             tc.tile_pool(name="db", bufs=2) as dbp, \
             tc.tile_pool(name="exp", bufs=3) as expp, \
             tc.tile_pool(name="sm", bufs=2) as smp, \
             tc.tile_pool(name="S", bufs=2, space="PSUM") as Sp, \
             tc.tile_pool(name="zp", bufs=2, space="PSUM") as zpp, \
             tc.tile_pool(name="rsp", bufs=1, space="PSUM") as rsp, \
             tc.tile_pool(name="gzp", bufs=1, space="PSUM") as gzp:

          # constants: loaded once, read-only across reps
          cst = bigp.tile([CI, 8], dt.float32, tag="cst")
          nc.sync.dma_start(cst[:], consts[:])
          ones_b = bigp.tile([128, 128], dt.bfloat16, tag="ones_b")
          nc.gpsimd.memset(ones_b[:], 1.0)

          pending_tail = [None]

          def emit_tail(cco, z_sb_r):
              # post-collective BN finish for a completed rep: gather stats,
              # compute scale/bias, apply, store
              stat_ag = dbp.tile([128, 16], dt.float32, tag="stat_ag")
              # cco is [8*128, 2] (shards along axis 0); gather to [p, (s c)]
              ag_view = cco.ap()[:].rearrange("(s p) c -> p s c", s=8)
              nc.sync.dma_start(stat_ag[:].rearrange("p (s c) -> p s c", c=2),
                                ag_view)
              agv = stat_ag[:].rearrange("p (s c) -> p s c", c=2)
              ag4 = dbp.tile([128, 8], dt.float32, tag="ag4")
              ag4v = ag4[:].rearrange("p (s c) -> p s c", c=2)
              nc.vector.tensor_add(ag4v[:], agv[:, 0:4, :], agv[:, 4:8, :])
              ag2 = dbp.tile([128, 4], dt.float32, tag="ag2")
              ag2v = ag2[:].rearrange("p (s c) -> p s c", c=2)
              nc.vector.tensor_add(ag2v[:], ag4v[:, 0:2, :], ag4v[:, 2:4, :])
              stat_all = dbp.tile([128, 2], dt.float32, tag="stat_all")
              nc.vector.tensor_add(stat_all[:], ag2v[:, 0, :], ag2v[:, 1, :])

              # mean = S1/NTOT ; ex2 = S2/NTOT ; var = ex2 - mean^2
              # rstd = exp(-0.5*ln(var+eps)) -- stays in the exp ACT table set
              me = dbp.tile([128, 2], dt.float32, tag="me")
              nc.vector.tensor_scalar_mul(me[:], stat_all[:], cst[:, 5:6])
              mean = me[:, 0:1]
              msq = dbp.tile([128, 1], dt.float32, tag="msq")
              nc.vector.tensor_mul(msq[:], mean, mean)
              var = dbp.tile([128, 1], dt.float32, tag="var")
              nc.vector.tensor_sub(var[:], me[:, 1:2], msq[:])
              lnv = dbp.tile([128, 1], dt.float32, tag="lnv")
              nc.scalar.activation(lnv[:], var[:], AF.Ln, bias=cst[:, 4:5])
              rstd = dbp.tile([128, 1], dt.float32, tag="rstd")
              nc.scalar.activation(rstd[:], lnv[:], AF.Exp, scale=-0.5)
              scale_t = dbp.tile([128, 1], dt.float32, tag="scale")
              nc.vector.tensor_mul(scale_t[:], rstd[:], cst[:, 1:2])
              mscale = dbp.tile([128, 1], dt.float32, tag="mscale")
              nc.vector.tensor_mul(mscale[:], mean, scale_t[:])
              bias2 = dbp.tile([128, 1], dt.float32, tag="bias2")
              nc.vector.tensor_sub(bias2[:], cst[:, 2:3], mscale[:])

              # apply (GPSIMD mul+add; DVE is busier) + store, split for
              # compute/DMA overlap
              zfin = dbp.tile([128, NI], dt.float32, tag="zfin")
              for h in range(4):
                  cs = slice(h * (NI // 4), (h + 1) * (NI // 4))
                  nc.gpsimd.tensor_scalar_mul(zfin[:, cs], z_sb_r[:, cs],
                                              scale_t[:])
                  nc.gpsimd.tensor_scalar_add(zfin[:, cs], zfin[:, cs],
                                              bias2[:])
                  nc.sync.dma_start(zout_d[:, cs], zfin[:, cs])

          for _rep in range(unroll):
              # ---- DRAM loads, ordered for earliest compute start ----
              wtT_r = [ldp.tile([128, CI], dt.float16, tag=f"wt{c}", name=f"wtT_r{c}") for c in range(2)]
              wpT_r = [ldp.tile([128, CI], dt.float16, tag=f"wp{c}", name=f"wpT_r{c}") for c in range(2)]
              wzgT_r = [ldp.tile([128, CI], dt.float16, tag=f"wz{c}", name=f"wzgT_r{c}") for c in range(2)]
              for c in range(2):
                  nc.sync.dma_start(wtT_r[c][:], wtT_d[c * 128:(c + 1) * 128, :])
                  nc.sync.dma_start(wpT_r[c][:], wpT_d[c * 128:(c + 1) * 128, :])
                  nc.sync.dma_start(wzgT_r[c][:], wzgT_d[c * 128:(c + 1) * 128, :])

              xo_r = [ldp.tile([128, NI], dt.float16, tag=f"xo{c}", name=f"xo_r{c}") for c in range(2)]
              for c in range(2):
                  nc.sync.dma_start(xo_r[c][:, 0:512], xob[c * 128:(c + 1) * 128, 0:512])
              xt_r = [ldp.tile([128, NJ], dt.float16, tag=f"xt{c}", name=f"xt_r{c}") for c in range(2)]
              for jc in range(NJ // 1024):
                  cs = slice(jc * 1024, (jc + 1) * 1024)
                  for c in range(2):
                      nc.sync.dma_start(xt_r[c][:, cs], xtb[c * 128:(c + 1) * 128, cs])
              for c in range(2):
                  nc.sync.dma_start(xo_r[c][:, 512:NI], xob[c * 128:(c + 1) * 128, 512:NI])

              # ---- projections ----
              phi = dbp.tile([128, NI], dt.float16, tag="phi")
              ps = Sp.tile([128, 512], dt.float32, tag="S", name="ps_phi0")
              for c in range(2):
                  nc.tensor.matmul(ps[:], wpT_r[c][:], xo_r[c][:, 0:512],
                                   start=(c == 0), stop=(c == 1))
              nc.vector.tensor_scalar_add(phi[:, 0:512], ps[:], cst[:, 0:1])

              theta = dbp.tile([128, NJ], dt.float16, tag="theta")
              for jc in range(NJ // 512):
                  ps = Sp.tile([128, 512], dt.float32, tag="S", name="ps_th")
                  for c in range(2):
                      nc.tensor.matmul(ps[:], wtT_r[c][:],
                                       xt_r[c][:, jc * 512:(jc + 1) * 512],
                                       start=(c == 0), stop=(c == 1))
                  # theta bias is constant per softmax row -> cancels; plain copy
                  nc.vector.tensor_copy(theta[:, jc * 512:(jc + 1) * 512], ps[:])

              z_sb = dbp.tile([128, NI], dt.float32, tag="z_sb")
              gzT = dbp.tile([128, NJ], dt.bfloat16, tag="gzT")

              e_chunks = {}

              def emit_scores_group(k, g0, gn, interleave=None):
                  pk = slice(k * IC, (k + 1) * IC)
                  e3 = e_chunks[k][:].rearrange("p (i j) -> p j i", j=JBS)
                  S_ps = Sp.tile([128, 1024], dt.float32, tag="S", name="S_ps")
                  for jj in range(gn):
                      jb = g0 + jj
                      nc.tensor.matmul(S_ps[:, jj * IC:(jj + 1) * IC],
                                       theta[:, jb * 128:(jb + 1) * 128],
                                       phi[:, pk], start=True, stop=True)
                      if interleave is not None:
                          interleave(jb)
                  nc.scalar.activation(
                      e3[:, g0:g0 + gn, :],
                      S_ps[:].rearrange("p (j i) -> p j i", i=IC)[:, 0:gn, :],
                      AF.Exp, bias=cst[:, 3:4])

              def emit_gzT_block(jb):
                  ps = gzp.tile([128, 128], dt.float32, tag="gz", name="ps_gz")
                  for c in range(2):
                      nc.tensor.matmul(ps[:], xt_r[c][:, jb * 128:(jb + 1) * 128],
                                       wzgT_r[c][:], start=(c == 0), stop=(c == 1))
                  nc.vector.tensor_copy(gzT[:, jb * 128:(jb + 1) * 128], ps[:])

              # hoisted chunk 0: scores interleaved with gzT projection +
              # remaining phi chunks
              def chunk0_phi_chunk(jb):
                  # phi chunks 1..3 (cols 512..2048)
                  ps = Sp.tile([128, 512], dt.float32, tag="S", name="ps_phi")
                  cs = slice((jb + 1) * 512, (jb + 2) * 512)
                  for c in range(2):
                      nc.tensor.matmul(ps[:], wpT_r[c][:], xo_r[c][:, cs],
                                       start=(c == 0), stop=(c == 1))
                  nc.vector.tensor_scalar_add(phi[:, cs], ps[:], cst[:, 0:1])

              def chunk0_interleave(jb):
                  emit_gzT_block(jb)
                  if jb < 3:
                      chunk0_phi_chunk(jb)

              e_chunks[0] = expp.tile([128, IC * JBS], dt.bfloat16,
                                      tag="e", name="e0")
              g0 = 0
              for g, gn in enumerate(JGROUPS):
                  emit_scores_group(0, g0, gn, interleave=chunk0_interleave)
                  g0 += gn

              stat = dbp.tile([128, 2], dt.float32, tag="stat")

              def make_zprime_interleave(km1):
                  # one z'(chunk km1) matmul per score matmul of chunk km1+1
                  e3m = e_chunks[km1][:].rearrange("p (i j) -> p j i", j=JBS)
                  zpart = zpp.tile([128, IC], dt.float32, tag="zp", name="zpart")
                  st = {'m': 0}

                  def cb(_jb_score):
                      m = st['m']
                      st['m'] += 1
                      nc.tensor.matmul(zpart[:], gzT[:, m * 128:(m + 1) * 128],
                                       e3m[:, m, :], start=(m == 0),
                                       stop=(m == JBS - 1))
                  return cb, zpart

              def emit_chunk_reduce(k, zpart):
                  # softmax sums over jb for chunk k (DVE packed-bf16 tree for
                  # jb<24, GPSIMD tree for jb>=24), normalize + BN stat partials
                  pk = slice(k * IC, (k + 1) * IC)
                  ei = e_chunks[k][:].rearrange("p (i j) -> p i j", j=JBS)
                  t12 = smp.tile([128, IC * 12], dt.bfloat16, tag="t12", bufs=1)
                  t12v = t12[:].rearrange("p (i j) -> p i j", j=12)
                  nc.vector.tensor_add(t12v[:], ei[:, :, 0:12], ei[:, :, 12:24])
                  t6 = smp.tile([128, IC * 6], dt.bfloat16, tag="t6", bufs=1)
                  t6v = t6[:].rearrange("p (i j) -> p i j", j=6)
                  nc.vector.tensor_add(t6v[:], t12v[:, :, 0:6], t12v[:, :, 6:12])
                  t3 = smp.tile([128, IC * 3], dt.bfloat16, tag="t3", bufs=1)
                  t3v = t3[:].rearrange("p (i j) -> p i j", j=3)
                  nc.vector.tensor_add(t3v[:], t6v[:, :, 0:3], t6v[:, :, 3:6])
                  tx = smp.tile([128, IC], dt.float32, tag="tx", bufs=1)
                  nc.vector.tensor_add(tx[:], t3v[:, :, 0], t3v[:, :, 1])
                  s_bD = smp.tile([128, IC], dt.float32, tag="sbD", bufs=1)
                  nc.vector.tensor_add(s_bD[:], tx[:], t3v[:, :, 2])

                  u4 = smp.tile([128, IC * 4], dt.bfloat16, tag="u4", bufs=1)
                  u4v = u4[:].rearrange("p (i j) -> p i j", j=4)
                  nc.gpsimd.tensor_add(u4v[:], ei[:, :, 24:28], ei[:, :, 28:32])
                  u2 = smp.tile([128, IC * 2], dt.bfloat16, tag="u2", bufs=1)
                  u2v = u2[:].rearrange("p (i j) -> p i j", j=2)
                  nc.gpsimd.tensor_add(u2v[:], u4v[:, :, 0:2], u4v[:, :, 2:4])
                  s_bP = smp.tile([128, IC], dt.float32, tag="sbP", bufs=1)
                  nc.gpsimd.tensor_add(s_bP[:], u2v[:, :, 0], u2v[:, :, 1])

                  s_part = smp.tile([128, IC], dt.bfloat16, tag="sp")
                  nc.vector.tensor_add(s_part[:], s_bD[:], s_bP[:])
                  rs = rsp.tile([128, IC], dt.float32, tag="rs")
                  nc.tensor.matmul(rs[:], ones_b[:], s_part[:], start=True,
                                   stop=True)
                  rrs = smp.tile([128, IC], dt.float32, tag="rrs")
                  nc.vector.reciprocal(rrs[:], rs[:])

                  nc.vector.tensor_mul(z_sb[:, pk], zpart[:], rrs[:])
                  s1c = smp.tile([128, 1], dt.float32, tag="s1c")
                  nc.vector.reduce_sum(s1c[:], z_sb[:, pk],
                                       axis=mybir.AxisListType.X)
                  sq = smp.tile([128, IC], dt.float32, tag="sq")
                  nc.vector.tensor_mul(sq[:], z_sb[:, pk], z_sb[:, pk])
                  s2c = smp.tile([128, 1], dt.float32, tag="s2c")
                  nc.vector.reduce_sum(s2c[:], sq[:], axis=mybir.AxisListType.X)
                  if k == 0:
                      nc.vector.tensor_copy(stat[:, 0:1], s1c[:])
                      nc.vector.tensor_copy(stat[:, 1:2], s2c[:])
                  else:
                      nc.vector.tensor_add(stat[:, 0:1], stat[:, 0:1], s1c[:])
                      nc.vector.tensor_add(stat[:, 1:2], stat[:, 1:2], s2c[:])

              # chunks 1..7: score groups interleave the previous chunk's z'
              # matmuls so ACT never starves and PE never stalls on S_ps
              for k in range(1, NCH):
                  e_chunks[k] = expp.tile([128, IC * JBS], dt.bfloat16,
                                          tag="e", name="e")
                  cb, zpart_prev = make_zprime_interleave(k - 1)
                  g0 = 0
                  for g, gn in enumerate(JGROUPS):
                      emit_scores_group(k, g0, gn, interleave=cb)
                      g0 += gn
                  emit_chunk_reduce(k - 1, zpart_prev)
                  if k == 4 and pending_tail[0] is not None:
                      # previous rep's post-collective tail: by now its
                      # AllGather (issued a half-rep ago) has completed, so
                      # none of these ops head-of-line-block their queues
                      emit_tail(*pending_tail[0])
                      pending_tail[0] = None

              # chunk 7's z' runs unhidden at the rep tail
              e3m = e_chunks[NCH - 1][:].rearrange("p (i j) -> p j i", j=JBS)
              zpart = zpp.tile([128, IC], dt.float32, tag="zp", name="zpart7")
              for jb in range(JBS):
                  nc.tensor.matmul(zpart[:], gzT[:, jb * 128:(jb + 1) * 128],
                                   e3m[:, jb, :], start=(jb == 0),
                                   stop=(jb == JBS - 1))
              emit_chunk_reduce(NCH - 1, zpart)

              # ---- BN stats AllGather (issue only; tail is deferred to the
              # middle of the next rep so its waits never block the queues) ----
              cci = cc_in[_rep % 2]
              cco = cc_out[_rep % 2]
              nc.sync.dma_start(cci.ap()[:], stat[:])
              nc.gpsimd.collective_compute(
                  "AllGather", mybir.AluOpType.bypass,
                  replica_groups=[list(range(8))],
                  ins=[cci.ap().opt()], outs=[cco.ap().opt()])
              pending_tail[0] = (cco, z_sb)

          emit_tail(*pending_tail[0])
          pending_tail[0] = None

    nc.compile()

    return nc


def _prep_in_maps(inputs):
    f16 = np.float16
    xt_full = inputs['x_thisBranch'].reshape(B, CIN, NJ).astype(f16)
    xo_full = inputs['x_otherBranch'].reshape(B, CIN, NJ).astype(f16)
    wtT = np.ascontiguousarray(inputs['w_theta'].T.astype(f16))
    wpT = np.ascontiguousarray(inputs['w_phi'].T.astype(f16))
    w_zg = (inputs['w_z'].astype(np.float64) @ inputs['w_g'].astype(np.float64))
    wzgT = np.ascontiguousarray(w_zg.T.astype(f16))
    consts = np.zeros((CI, 8), np.float32)
    consts[:, 0] = inputs['b_phi']
    consts[:, 1] = inputs['bn_gamma']
    consts[:, 2] = inputs['bn_beta']
    consts[:, 3] = SHIFT
    consts[:, 4] = BN_EPS
    consts[:, 5] = 1.0 / NTOT
    in_maps = []
    for c in range(8):
        b, h = c // 2, c % 2
        in_maps.append({
            "xtb": np.ascontiguousarray(xt_full[b]),
            "xob": np.ascontiguousarray(xo_full[b][:, h * NI:(h + 1) * NI]),
            "wtT": wtT, "wpT": wpT, "wzgT": wzgT, "consts": consts,
        })
    return in_maps


def kernel(**inputs):
    from concourse.bass_utils import run_bass_kernel_spmd
    if "nc" not in _CACHE:
        _CACHE["nc"] = _build()
    nc = _CACHE["nc"]
    in_maps = _prep_in_maps(inputs)
    res = run_bass_kernel_spmd(nc, in_maps, list(range(8)))
    out = np.empty((B, CI, NJ), np.float32)
    for c in range(8):
        b, h = c // 2, c % 2
        out[b][:, h * NI:(h + 1) * NI] = res.results[c]["z"]
    return out.reshape(B, CI, H, W)


if __name__ == "__main__":
    inputs = np.load('/tmp/ref_inputs.npy', allow_pickle=True).item()
    ref = np.load('/tmp/ref_output.npy')
    got = kernel(**inputs)
    err = np.abs(got - ref)
    denom = np.abs(ref).max()
    print(f"abs max err: {err.max():.4e}  (ref absmax {denom:.3f})")
    print(f"Relative error: {err.max() / denom:.4e}")


# revision 18
# speedup vs baseline: 1.0023x; 1.0023x over previous
"""NLBlockND multi-cross attention block on 8 Trainium2 NeuronCores.

Per-core shard: core c handles batch b = c//2, spatial half h = c%2
(i in [h*2048, (h+1)*2048)).  Mixed 16-bit datapath: the score path (x, weights,
theta, phi) runs fp16 (10-bit mantissa, scores need the accuracy); the
exp/e path (e, gzT, softmax tree, ones) runs bf16 (e spans ~90 e-folds,
needs the exponent range).  Full PE rate at any moving width + FWL fast
weight loads for both; PSUM/stats fp32.  Softmax uses a constant shift of -50 which cancels in the ratio;
the theta conv bias is constant per softmax row so it cancels exactly
and is dropped; phi's bias is folded into the PSUM drain.  b_g/b_z drop
out in training-mode BN; w_z is folded into w_g on the host.  Softmax
j-reduction: 24 j-blocks tree-reduced on DVE (packed fp16, 2x mode),
8 on GPSIMD.  BN rstd = exp(-0.5*ln(var+eps)) so every activation stays
in one ACT table set (no reloads).  Cross-rep tiles (theta/phi/gzT/
z_sb/stat) are double-buffered so one rep's BN-stats AllGather + apply
tail overlaps the next rep's compute.  BN batch stats are all-gathered
([128,2]) across the 8 cores.
"""
import sys
sys.path.insert(0, '/opt/trn_rl_repo')

import numpy as np

B, CIN, CI, H, W = 4, 256, 128, 64, 64
NJ = H * W              # 4096 (full spatial, j axis)
NI = NJ // 2            # 2048 per-core i positions
IC = 256                # i-chunk
NCH = NI // IC          # 8 chunks
JBS = NJ // 128         # 32 j-blocks
JGROUPS = [4] * 8       # j-blocks per exp group (PSUM: 2 banks)
MD = 24                 # j-blocks tree-reduced on DVE; JBS-MD on GPSIMD
SHIFT = -50.0
BN_EPS = 1e-5
NTOT = float(B * NJ)    # BN count per channel

_CACHE = {}


def _build(unroll=1):
    import concourse.bacc as bacc
    import concourse.mybir as mybir
    from concourse import tile

    dt = mybir.dt
    AF = mybir.ActivationFunctionType
    ALU = mybir.AluOpType

    nc = bacc.Bacc("TRN2", target_bir_lowering=False, debug=False, num_devices=8)

    xtb = nc.dram_tensor("xtb", [CIN, NJ], dt.float16, kind="ExternalInput").ap()
    xob = nc.dram_tensor("xob", [CIN, NI], dt.float16, kind="ExternalInput").ap()
    wtT_d = nc.dram_tensor("wtT", [CIN, CI], dt.float16, kind="ExternalInput").ap()
    wpT_d = nc.dram_tensor("wpT", [CIN, CI], dt.float16, kind="ExternalInput").ap()
    wzgT_d = nc.dram_tensor("wzgT", [CIN, CI], dt.float16, kind="ExternalInput").ap()
    # consts columns: 0 b_phi, 1 gamma, 2 beta, 3 SHIFT, 4 eps, 5 1/NTOT
    consts = nc.dram_tensor("consts", [CI, 8], dt.float32, kind="ExternalInput").ap()
    zout_d = nc.dram_tensor("z", [CI, NI], dt.float32, kind="ExternalOutput").ap()

    cc_in = [nc.dram_tensor(f"cc_in{i}", [CI, 2], dt.float32) for i in range(2)]
    cc_out = [nc.dram_tensor(f"cc_out{i}", [8 * CI, 2], dt.float32,
                             addr_space="Shared") for i in range(2)]

    with tile.TileContext(nc) as tc, \
         nc.allow_low_precision(reason="fp16 datapath, fp32 accumulation"):
        with tc.tile_pool(name="ld", bufs=1) as ldp, \
             tc.tile_pool(name="big", bufs=1) as bigp, \
             tc.tile_pool(name="db", bufs=2) as dbp, \
             tc.tile_pool(name="exp", bufs=3) as expp, \
             tc.tile_pool(name="sm", bufs=2) as smp, \
             tc.tile_pool(name="S", bufs=2, space="PSUM") as Sp, \
             tc.tile_pool(name="zp", bufs=2, space="PSUM") as zpp, \
             tc.tile_pool(name="rsp", bufs=1, space="PSUM") as rsp, \
             tc.tile_pool(name="gzp", bufs=1, space="PSUM") as gzp:

          # constants: loaded once, read-only across reps
          cst = bigp.tile([CI, 8], dt.float32, tag="cst")
          nc.sync.dma_start(cst[:], consts[:])
          ones_b = bigp.tile([128, 128], dt.bfloat16, tag="ones_b")
          nc.gpsimd.memset(ones_b[:], 1.0)

          pending_tail = [None]

          def emit_tail(cco, z_sb_r):
              # post-collective BN finish for a completed rep: gather stats,
              # compute scale/bias, apply, store
              stat_ag = dbp.tile([128, 16], dt.float32, tag="stat_ag")
              # cco is [8*128, 2] (shards along axis 0); gather to [p, (s c)]
              ag_view = cco.ap()[:].rearrange("(s p) c -> p s c", s=8)
              nc.sync.dma_start(stat_ag[:].rearrange("p (s c) -> p s c", c=2),
                                ag_view)
              agv = stat_ag[:].rearrange("p (s c) -> p s c", c=2)
              ag4 = dbp.tile([128, 8], dt.float32, tag="ag4")
              ag4v = ag4[:].rearrange("p (s c) -> p s c", c=2)
              nc.vector.tensor_add(ag4v[:], agv[:, 0:4, :], agv[:, 4:8, :])
              ag2 = dbp.tile([128, 4], dt.float32, tag="ag2")
              ag2v = ag2[:].rearrange("p (s c) -> p s c", c=2)
              nc.vector.tensor_add(ag2v[:], ag4v[:, 0:2, :], ag4v[:, 2:4, :])
              stat_all = dbp.tile([128, 2], dt.float32, tag="stat_all")
              nc.vector.tensor_add(stat_all[:], ag2v[:, 0, :], ag2v[:, 1, :])

              # mean = S1/NTOT ; ex2 = S2/NTOT ; var = ex2 - mean^2
              # rstd = exp(-0.5*ln(var+eps)) -- stays in the exp ACT table set
              me = dbp.tile([128, 2], dt.float32, tag="me")
              nc.vector.tensor_scalar_mul(me[:], stat_all[:], cst[:, 5:6])
              mean = me[:, 0:1]
              msq = dbp.tile([128, 1], dt.float32, tag="msq")
              nc.vector.tensor_mul(msq[:], mean, mean)
              var = dbp.tile([128, 1], dt.float32, tag="var")
              nc.vector.tensor_sub(var[:], me[:, 1:2], msq[:])
              lnv = dbp.tile([128, 1], dt.float32, tag="lnv")
              nc.scalar.activation(lnv[:], var[:], AF.Ln, bias=cst[:, 4:5])
              rstd = dbp.tile([128, 1], dt.float32, tag="rstd")
              nc.scalar.activation(rstd[:], lnv[:], AF.Exp, scale=-0.5)
              scale_t = dbp.tile([128, 1], dt.float32, tag="scale")
              nc.vector.tensor_mul(scale_t[:], rstd[:], cst[:, 1:2])
              mscale = dbp.tile([128, 1], dt.float32, tag="mscale")
              nc.vector.tensor_mul(mscale[:], mean, scale_t[:])
              bias2 = dbp.tile([128, 1], dt.float32, tag="bias2")
              nc.vector.tensor_sub(bias2[:], cst[:, 2:3], mscale[:])

              # apply (GPSIMD mul+add; DVE is busier) + store, split for
              # compute/DMA overlap
              zfin = dbp.tile([128, NI], dt.float32, tag="zfin")
              for h in range(4):
                  cs = slice(h * (NI // 4), (h + 1) * (NI // 4))
                  nc.gpsimd.tensor_scalar_mul(zfin[:, cs], z_sb_r[:, cs],
                                              scale_t[:])
                  nc.gpsimd.tensor_scalar_add(zfin[:, cs], zfin[:, cs],
                                              bias2[:])
                  nc.sync.dma_start(zout_d[:, cs], zfin[:, cs])

          def emit_loads():
              # ---- DRAM loads, ordered for earliest compute start ----
              L = {}
              L['wt'] = [ldp.tile([128, CI], dt.float16, tag=f"wt{c}", name=f"wtT_r{c}") for c in range(2)]
              L['wp'] = [ldp.tile([128, CI], dt.float16, tag=f"wp{c}", name=f"wpT_r{c}") for c in range(2)]
              L['wzg'] = [ldp.tile([128, CI], dt.float16, tag=f"wz{c}", name=f"wzgT_r{c}") for c in range(2)]
              for c in range(2):
                  nc.sync.dma_start(L['wt'][c][:], wtT_d[c * 128:(c + 1) * 128, :])
                  nc.sync.dma_start(L['wp'][c][:], wpT_d[c * 128:(c + 1) * 128, :])
                  nc.sync.dma_start(L['wzg'][c][:], wzgT_d[c * 128:(c + 1) * 128, :])
              L['xo'] = [ldp.tile([128, NI], dt.float16, tag=f"xo{c}", name=f"xo_r{c}") for c in range(2)]
              for c in range(2):
                  nc.sync.dma_start(L['xo'][c][:, 0:512], xob[c * 128:(c + 1) * 128, 0:512])
              L['xt'] = [ldp.tile([128, NJ], dt.float16, tag=f"xt{c}", name=f"xt_r{c}") for c in range(2)]
              for jc in range(NJ // 1024):
                  cs = slice(jc * 1024, (jc + 1) * 1024)
                  for c in range(2):
                      nc.sync.dma_start(L['xt'][c][:, cs], xtb[c * 128:(c + 1) * 128, cs])
              for c in range(2):
                  nc.sync.dma_start(L['xo'][c][:, 512:NI], xob[c * 128:(c + 1) * 128, 512:NI])
              return L

          loads_cur = emit_loads()
          pre = {}

          for _rep in range(unroll):
              L = loads_cur
              wtT_r, wpT_r, wzgT_r = L['wt'], L['wp'], L['wzg']
              xo_r, xt_r = L['xo'], L['xt']

              # ---- projections (skipped when pre-projected last rep) ----
              if 'phi' in pre:
                  phi = pre.pop('phi')
              else:
                  phi = dbp.tile([128, NI], dt.float16, tag="phi")
                  ps = Sp.tile([128, 512], dt.float32, tag="S", name="ps_phi0")
                  for c in range(2):
                      nc.tensor.matmul(ps[:], wpT_r[c][:], xo_r[c][:, 0:512],
                                       start=(c == 0), stop=(c == 1))
                  nc.vector.tensor_scalar_add(phi[:, 0:512], ps[:], cst[:, 0:1])

              if 'theta' in pre:
                  theta = pre.pop('theta')
              else:
                  theta = dbp.tile([128, NJ], dt.float16, tag="theta")
                  for jc in range(NJ // 512):
                      ps = Sp.tile([128, 512], dt.float32, tag="S", name="ps_th")
                      for c in range(2):
                          nc.tensor.matmul(ps[:], wtT_r[c][:],
                                           xt_r[c][:, jc * 512:(jc + 1) * 512],
                                           start=(c == 0), stop=(c == 1))
                      # theta bias is constant per softmax row -> cancels
                      nc.vector.tensor_copy(theta[:, jc * 512:(jc + 1) * 512], ps[:])

              z_sb = dbp.tile([128, NI], dt.float32, tag="z_sb")
              gzT = dbp.tile([128, NJ], dt.bfloat16, tag="gzT")

              e_chunks = {}

              def emit_scores_group(k, g0, gn, interleave=None):
                  pk = slice(k * IC, (k + 1) * IC)
                  e3 = e_chunks[k][:].rearrange("p (i j) -> p j i", j=JBS)
                  S_ps = Sp.tile([128, 1024], dt.float32, tag="S", name="S_ps")
                  for jj in range(gn):
                      jb = g0 + jj
                      nc.tensor.matmul(S_ps[:, jj * IC:(jj + 1) * IC],
                                       theta[:, jb * 128:(jb + 1) * 128],
                                       phi[:, pk], start=True, stop=True)
                      if interleave is not None:
                          interleave(jb)
                  nc.scalar.activation(
                      e3[:, g0:g0 + gn, :],
                      S_ps[:].rearrange("p (j i) -> p j i", i=IC)[:, 0:gn, :],
                      AF.Exp, bias=cst[:, 3:4])

              def emit_gzT_block(jb):
                  ps = gzp.tile([128, 128], dt.float32, tag="gz", name="ps_gz")
                  for c in range(2):
                      nc.tensor.matmul(ps[:], xt_r[c][:, jb * 128:(jb + 1) * 128],
                                       wzgT_r[c][:], start=(c == 0), stop=(c == 1))
                  nc.vector.tensor_copy(gzT[:, jb * 128:(jb + 1) * 128], ps[:])

              # hoisted chunk 0: scores interleaved with gzT projection +
              # remaining phi chunks
              def chunk0_phi_chunk(jb):
                  # phi chunks 1..3 (cols 512..2048)
                  ps = Sp.tile([128, 512], dt.float32, tag="S", name="ps_phi")
                  cs = slice((jb + 1) * 512, (jb + 2) * 512)
                  for c in range(2):
                      nc.tensor.matmul(ps[:], wpT_r[c][:], xo_r[c][:, cs],
                                       start=(c == 0), stop=(c == 1))
                  nc.vector.tensor_scalar_add(phi[:, cs], ps[:], cst[:, 0:1])

              def chunk0_interleave(jb):
                  emit_gzT_block(jb)
                  if jb < 3:
                      chunk0_phi_chunk(jb)

              e_chunks[0] = expp.tile([128, IC * JBS], dt.bfloat16,
                                      tag="e", name="e0")
              g0 = 0
              for g, gn in enumerate(JGROUPS):
                  emit_scores_group(0, g0, gn, interleave=chunk0_interleave)
                  g0 += gn

              stat = dbp.tile([128, 2], dt.float32, tag="stat")

              def make_zprime_interleave(km1):
                  # one z'(chunk km1) matmul per score matmul of chunk km1+1
                  e3m = e_chunks[km1][:].rearrange("p (i j) -> p j i", j=JBS)
                  zpart = zpp.tile([128, IC], dt.float32, tag="zp", name="zpart")
                  st = {'m': 0}

                  def cb(_jb_score):
                      m = st['m']
                      st['m'] += 1
                      nc.tensor.matmul(zpart[:], gzT[:, m * 128:(m + 1) * 128],
                                       e3m[:, m, :], start=(m == 0),
                                       stop=(m == JBS - 1))
                  return cb, zpart

              def emit_chunk_reduce(k, zpart):
                  # softmax sums over jb for chunk k (DVE packed-bf16 tree for
                  # jb<24, GPSIMD tree for jb>=24), normalize + BN stat partials
                  pk = slice(k * IC, (k + 1) * IC)
                  ei = e_chunks[k][:].rearrange("p (i j) -> p i j", j=JBS)
                  t12 = smp.tile([128, IC * 12], dt.bfloat16, tag="t12", bufs=1)
                  t12v = t12[:].rearrange("p (i j) -> p i j", j=12)
                  nc.vector.tensor_add(t12v[:], ei[:, :, 0:12], ei[:, :, 12:24])
                  t6 = smp.tile([128, IC * 6], dt.bfloat16, tag="t6", bufs=1)
                  t6v = t6[:].rearrange("p (i j) -> p i j", j=6)
                  nc.vector.tensor_add(t6v[:], t12v[:, :, 0:6], t12v[:, :, 6:12])
                  t3 = smp.tile([128, IC * 3], dt.bfloat16, tag="t3", bufs=1)
                  t3v = t3[:].rearrange("p (i j) -> p i j", j=3)
                  nc.vector.tensor_add(t3v[:], t6v[:, :, 0:3], t6v[:, :, 3:6])
                  tx = smp.tile([128, IC], dt.float32, tag="tx", bufs=1)
                  nc.vector.tensor_add(tx[:], t3v[:, :, 0], t3v[:, :, 1])
                  s_bD = smp.tile([128, IC], dt.float32, tag="sbD", bufs=1)
                  nc.vector.tensor_add(s_bD[:], tx[:], t3v[:, :, 2])

                  u4 = smp.tile([128, IC * 4], dt.bfloat16, tag="u4", bufs=1)
                  u4v = u4[:].rearrange("p (i j) -> p i j", j=4)
                  nc.gpsimd.tensor_add(u4v[:], ei[:, :, 24:28], ei[:, :, 28:32])
                  u2 = smp.tile([128, IC * 2], dt.bfloat16, tag="u2", bufs=1)
                  u2v = u2[:].rearrange("p (i j) -> p i j", j=2)
                  nc.gpsimd.tensor_add(u2v[:], u4v[:, :, 0:2], u4v[:, :, 2:4])
                  s_bP = smp.tile([128, IC], dt.float32, tag="sbP", bufs=1)
                  nc.gpsimd.tensor_add(s_bP[:], u2v[:, :, 0], u2v[:, :, 1])

                  s_part = smp.tile([128, IC], dt.bfloat16, tag="sp")
                  nc.vector.tensor_add(s_part[:], s_bD[:], s_bP[:])
                  rs = rsp.tile([128, IC], dt.float32, tag="rs")
                  nc.tensor.matmul(rs[:], ones_b[:], s_part[:], start=True,
                                   stop=True)
                  rrs = smp.tile([128, IC], dt.float32, tag="rrs")
                  nc.vector.reciprocal(rrs[:], rs[:])

                  nc.vector.tensor_mul(z_sb[:, pk], zpart[:], rrs[:])
                  s1c = smp.tile([128, 1], dt.float32, tag="s1c")
                  nc.vector.reduce_sum(s1c[:], z_sb[:, pk],
                                       axis=mybir.AxisListType.X)
                  sq = smp.tile([128, IC], dt.float32, tag="sq")
                  nc.vector.tensor_mul(sq[:], z_sb[:, pk], z_sb[:, pk])
                  s2c = smp.tile([128, 1], dt.float32, tag="s2c")
                  nc.vector.reduce_sum(s2c[:], sq[:], axis=mybir.AxisListType.X)
                  if k == 0:
                      nc.vector.tensor_copy(stat[:, 0:1], s1c[:])
                      nc.vector.tensor_copy(stat[:, 1:2], s2c[:])
                  else:
                      nc.vector.tensor_add(stat[:, 0:1], stat[:, 0:1], s1c[:])
                      nc.vector.tensor_add(stat[:, 1:2], stat[:, 1:2], s2c[:])

              # chunks 1..7: score groups interleave the previous chunk's z'
              # matmuls so ACT never starves and PE never stalls on S_ps.
              # ch6 additionally pre-projects the next rep's theta; ch7
              # straddles 24 of this rep's chunk-7 z' matmuls (each needs its
              # exp group done, which runs 2 groups ahead).
              prefetch = _rep + 1 < unroll
              loads_next = None
              theta_next = None
              zpart7 = None
              for k in range(1, NCH):
                  e_chunks[k] = expp.tile([128, IC * JBS], dt.bfloat16,
                                          tag="e", name="e")
                  cb, zpart_prev = make_zprime_interleave(k - 1)
                  if k == 6 and prefetch:
                      theta_next = dbp.tile([128, NJ], dt.float16,
                                            tag="theta", name="theta_next")
                      Ln = loads_next
                      st6 = {'m': 0, 'ps': None}
                      base_cb = cb

                      def cb(_jb, base_cb=base_cb, Ln=Ln, st6=st6,
                             theta_next=theta_next):
                          base_cb(_jb)
                          m = st6['m']
                          st6['m'] += 1
                          if m < 16:
                              return
                          tj, c = (m - 16) // 2, (m - 16) % 2
                          if c == 0:
                              st6['ps'] = Sp.tile([128, 512], dt.float32,
                                                  tag="S", name="ps_thn")
                          nc.tensor.matmul(
                              st6['ps'][:], Ln['wt'][c][:],
                              Ln['xt'][c][:, tj * 512:(tj + 1) * 512],
                              start=(c == 0), stop=(c == 1))
                          if c == 1:
                              nc.vector.tensor_copy(
                                  theta_next[:, tj * 512:(tj + 1) * 512],
                                  st6['ps'][:])
                  elif k == 7:
                      e3m7 = e_chunks[7][:].rearrange("p (i j) -> p j i", j=JBS)
                      zpart7 = zpp.tile([128, IC], dt.float32, tag="zp",
                                        name="zpart7")
                      st7 = {'m': 0}
                      base_cb = cb

                      def cb(_jb, base_cb=base_cb, st7=st7):
                          base_cb(_jb)
                          m = st7['m']
                          st7['m'] += 1
                          if m < 8:
                              return
                          jb7 = m - 8
                          nc.tensor.matmul(zpart7[:],
                                           gzT[:, jb7 * 128:(jb7 + 1) * 128],
                                           e3m7[:, jb7, :], start=(jb7 == 0),
                                           stop=False)
                  g0 = 0
                  for g, gn in enumerate(JGROUPS):
                      emit_scores_group(k, g0, gn, interleave=cb)
                      g0 += gn
                  emit_chunk_reduce(k - 1, zpart_prev)
                  if k == 4:
                      if prefetch:
                          loads_next = emit_loads()
                      if pending_tail[0] is not None:
                          # previous rep's post-collective tail: by now its
                          # AllGather (issued a half-rep ago) has completed, so
                          # none of these ops head-of-line-block their queues
                          emit_tail(*pending_tail[0])
                          pending_tail[0] = None

              # chunk 7's remaining z' matmuls (jb 24..31)
              e3m = e_chunks[NCH - 1][:].rearrange("p (i j) -> p j i", j=JBS)
              for jb in range(24, JBS):
                  nc.tensor.matmul(zpart7[:], gzT[:, jb * 128:(jb + 1) * 128],
                                   e3m[:, jb, :], start=False,
                                   stop=(jb == JBS - 1))
              emit_chunk_reduce(NCH - 1, zpart7)

              # next rep's phi cols 0:512
              if prefetch:
                  phi_next = dbp.tile([128, NI], dt.float16, tag="phi",
                                      name="phi_next")
                  ps = Sp.tile([128, 512], dt.float32, tag="S", name="ps_phn")
                  for c in range(2):
                      nc.tensor.matmul(ps[:], loads_next['wp'][c][:],
                                       loads_next['xo'][c][:, 0:512],
                                       start=(c == 0), stop=(c == 1))
                  nc.vector.tensor_scalar_add(phi_next[:, 0:512], ps[:],
                                              cst[:, 0:1])
                  pre = {'theta': theta_next, 'phi': phi_next}
                  loads_cur = loads_next

              # ---- BN stats AllGather (issue only; tail is deferred to the
              # middle of the next rep so its waits never block the queues) ----
              cci = cc_in[_rep % 2]
              cco = cc_out[_rep % 2]
              nc.sync.dma_start(cci.ap()[:], stat[:])
              nc.gpsimd.collective_compute(
                  "AllGather", mybir.AluOpType.bypass,
                  replica_groups=[list(range(8))],
                  ins=[cci.ap().opt()], outs=[cco.ap().opt()])
              pending_tail[0] = (cco, z_sb)

          emit_tail(*pending_tail[0])
          pending_tail[0] = None

    nc.compile()

    return nc


def _prep_in_maps(inputs):
    f16 = np.float16
    xt_full = inputs['x_thisBranch'].reshape(B, CIN, NJ).astype(f16)
    xo_full = inputs['x_otherBranch'].reshape(B, CIN, NJ).astype(f16)
    wtT = np.ascontiguousarray(inputs['w_theta'].T.astype(f16))
    wpT = np.ascontiguousarray(inputs['w_phi'].T.astype(f16))
    w_zg = (inputs['w_z'].astype(np.float64) @ inputs['w_g'].astype(np.float64))
    wzgT = np.ascontiguousarray(w_zg.T.astype(f16))
    consts = np.zeros((CI, 8), np.float32)
    consts[:, 0] = inputs['b_phi']
    consts[:, 1] = inputs['bn_gamma']
    consts[:, 2] = inputs['bn_beta']
    consts[:, 3] = SHIFT
    consts[:, 4] = BN_EPS
    consts[:, 5] = 1.0 / NTOT
    in_maps = []
    for c in range(8):
        b, h = c // 2, c % 2
        in_maps.append({
            "xtb": np.ascontiguousarray(xt_full[b]),
            "xob": np.ascontiguousarray(xo_full[b][:, h * NI:(h + 1) * NI]),
            "wtT": wtT, "wpT": wpT, "wzgT": wzgT, "consts": consts,
        })
    return in_maps


def kernel(**inputs):
    from concourse.bass_utils import run_bass_kernel_spmd
    if "nc" not in _CACHE:
        _CACHE["nc"] = _build()
    nc = _CACHE["nc"]
    in_maps = _prep_in_maps(inputs)
    res = run_bass_kernel_spmd(nc, in_maps, list(range(8)))
    out = np.empty((B, CI, NJ), np.float32)
    for c in range(8):
        b, h = c // 2, c % 2
        out[b][:, h * NI:(h + 1) * NI] = res.results[c]["z"]
    return out.reshape(B, CI, H, W)


if __name__ == "__main__":
    inputs = np.load('/tmp/ref_inputs.npy', allow_pickle=True).item()
    ref = np.load('/tmp/ref_output.npy')
    got = kernel(**inputs)
    err = np.abs(got - ref)
    denom = np.abs(ref).max()
    print(f"abs max err: {err.max():.4e}  (ref absmax {denom:.3f})")
    print(f"Relative error: {err.max() / denom:.4e}")


# revision 21
# speedup vs baseline: 1.5207x; 1.5172x over previous
"""NLBlockND multi-cross attention block on 8 Trainium2 NeuronCores.

Per-core shard: core c handles batch b = c//2, spatial half h = c%2
(i in [h*2048, (h+1)*2048)).  Mixed 16-bit datapath: the score path (x, weights,
theta, phi) runs fp16 (10-bit mantissa, scores need the accuracy); the
exp/e path (e, gzT, softmax tree, ones) runs bf16 (e spans ~90 e-folds,
needs the exponent range).  Full PE rate at any moving width + FWL fast
weight loads for both; PSUM/stats fp32.  Softmax uses a constant shift of -50 which cancels in the ratio;
the theta conv bias is constant per softmax row so it cancels exactly
and is dropped; phi's bias is folded into the PSUM drain.  b_g/b_z drop
out in training-mode BN; w_z is folded into w_g on the host.  Softmax
j-reduction: 24 j-blocks tree-reduced on DVE (packed fp16, 2x mode),
8 on GPSIMD.  BN rstd = exp(-0.5*ln(var+eps)) so every activation stays
in one ACT table set (no reloads).  Cross-rep tiles (theta/phi/gzT/
z_sb/stat) are double-buffered so one rep's BN-stats AllGather + apply
tail overlaps the next rep's compute.  BN batch stats are all-gathered
([128,2]) across the 8 cores.
"""
import sys
sys.path.insert(0, '/opt/trn_rl_repo')

import numpy as np

B, CIN, CI, H, W = 4, 256, 128, 64, 64
NJ = H * W              # 4096 (full spatial, j axis)
NI = NJ // 2            # 2048 per-core i positions
IC = 256                # i-chunk
NCH = NI // IC          # 8 chunks
JBS = NJ // 128         # 32 j-blocks
JGROUPS = [4] * 8       # j-blocks per exp group (PSUM: 2 banks)
MD = 24                 # j-blocks tree-reduced on DVE; JBS-MD on GPSIMD
SHIFT = -50.0
BN_EPS = 1e-5
NTOT = float(B * NJ)    # BN count per channel

_CACHE = {}


def _build(unroll=1):
    import concourse.bacc as bacc
    import concourse.mybir as mybir
    from concourse import tile

    dt = mybir.dt
    AF = mybir.ActivationFunctionType
    ALU = mybir.AluOpType

    nc = bacc.Bacc("TRN2", target_bir_lowering=False, debug=False, num_devices=8)

    xtb = nc.dram_tensor("xtb", [CIN, NJ], dt.float16, kind="ExternalInput").ap()
    xob = nc.dram_tensor("xob", [CIN, NI], dt.float16, kind="ExternalInput").ap()
    wtT_d = nc.dram_tensor("wtT", [CIN, CI], dt.float16, kind="ExternalInput").ap()
    wpT_d = nc.dram_tensor("wpT", [CIN, CI], dt.float16, kind="ExternalInput").ap()
    wzgT_d = nc.dram_tensor("wzgT", [CIN, CI], dt.float16, kind="ExternalInput").ap()
    # consts columns: 0 b_phi, 1 gamma, 2 beta, 3 SHIFT, 4 eps, 5 1/NTOT
    consts = nc.dram_tensor("consts", [CI, 8], dt.float32, kind="ExternalInput").ap()
    zout_d = nc.dram_tensor("z", [CI, NI], dt.float32, kind="ExternalOutput").ap()

    cc_in = [nc.dram_tensor(f"cc_in{i}", [CI, 2], dt.float32) for i in range(2)]
    cc_out = [nc.dram_tensor(f"cc_out{i}", [8 * CI, 2], dt.float32,
                             addr_space="Shared") for i in range(2)]

    with tile.TileContext(nc) as tc, \
         nc.allow_low_precision(reason="fp16 datapath, fp32 accumulation"):
        with tc.tile_pool(name="ld", bufs=1) as ldp, \
             tc.tile_pool(name="big", bufs=1) as bigp, \
             tc.tile_pool(name="db", bufs=2) as dbp, \
             tc.tile_pool(name="exp", bufs=3) as expp, \
             tc.tile_pool(name="sm", bufs=2) as smp, \
             tc.tile_pool(name="S", bufs=2, space="PSUM") as Sp, \
             tc.tile_pool(name="zp", bufs=2, space="PSUM") as zpp, \
             tc.tile_pool(name="rsp", bufs=1, space="PSUM") as rsp, \
             tc.tile_pool(name="gzp", bufs=1, space="PSUM") as gzp:

          # constants: loaded once, read-only across reps
          cst = bigp.tile([CI, 8], dt.float32, tag="cst")
          nc.sync.dma_start(cst[:], consts[:])
          ones_b = bigp.tile([128, 128], dt.bfloat16, tag="ones_b")
          nc.gpsimd.memset(ones_b[:], 1.0)

          pending_tail = [None]
          tail_mv = [None]
          tail_rstd = [None]

          def emit_tail_stats(cco):
              # post-collective stage 1: gather stats, reduce to mean/var
              stat_ag = dbp.tile([128, 16], dt.float32, tag="stat_ag")
              # cco is [8*128, 2] (shards along axis 0); gather to [p, (s c)]
              ag_view = cco.ap()[:].rearrange("(s p) c -> p s c", s=8)
              nc.sync.dma_start(stat_ag[:].rearrange("p (s c) -> p s c", c=2),
                                ag_view)
              agv = stat_ag[:].rearrange("p (s c) -> p s c", c=2)
              ag4 = dbp.tile([128, 8], dt.float32, tag="ag4")
              ag4v = ag4[:].rearrange("p (s c) -> p s c", c=2)
              nc.vector.tensor_add(ag4v[:], agv[:, 0:4, :], agv[:, 4:8, :])
              ag2 = dbp.tile([128, 4], dt.float32, tag="ag2")
              ag2v = ag2[:].rearrange("p (s c) -> p s c", c=2)
              nc.vector.tensor_add(ag2v[:], ag4v[:, 0:2, :], ag4v[:, 2:4, :])
              stat_all = dbp.tile([128, 2], dt.float32, tag="stat_all")
              nc.vector.tensor_add(stat_all[:], ag2v[:, 0, :], ag2v[:, 1, :])

              # mean = S1/NTOT ; ex2 = S2/NTOT ; var = ex2 - mean^2
              me = dbp.tile([128, 2], dt.float32, tag="me")
              nc.vector.tensor_scalar_mul(me[:], stat_all[:], cst[:, 5:6])
              mean = me[:, 0:1]
              msq = dbp.tile([128, 1], dt.float32, tag="msq")
              nc.vector.tensor_mul(msq[:], mean, mean)
              var = dbp.tile([128, 1], dt.float32, tag="var")
              nc.vector.tensor_sub(var[:], me[:, 1:2], msq[:])
              return me, var

          def emit_tail_rstd(me_var):
              # stage 2: rstd = exp(-0.5*ln(var+eps)) -- stays in the exp ACT
              # table set, and by now var is long ready so the ACT queue
              # never head-of-line-waits on the DVE chain
              me, var = me_var
              lnv = dbp.tile([128, 1], dt.float32, tag="lnv")
              nc.scalar.activation(lnv[:], var[:], AF.Ln, bias=cst[:, 4:5])
              rstd = dbp.tile([128, 1], dt.float32, tag="rstd")
              nc.scalar.activation(rstd[:], lnv[:], AF.Exp, scale=-0.5)
              return rstd

          def emit_tail_apply(me_var, rstd, z_sb_r):
              # stage 3: scale/bias then apply (GPSIMD; DVE is busier) + store
              me, var = me_var
              scale_t = dbp.tile([128, 1], dt.float32, tag="scale")
              nc.vector.tensor_mul(scale_t[:], rstd[:], cst[:, 1:2])
              mscale = dbp.tile([128, 1], dt.float32, tag="mscale")
              nc.vector.tensor_mul(mscale[:], me[:, 0:1], scale_t[:])
              bias2 = dbp.tile([128, 1], dt.float32, tag="bias2")
              nc.vector.tensor_sub(bias2[:], cst[:, 2:3], mscale[:])
              zfin = dbp.tile([128, NI], dt.float32, tag="zfin")
              for h in range(4):
                  cs = slice(h * (NI // 4), (h + 1) * (NI // 4))
                  nc.gpsimd.tensor_scalar_mul(zfin[:, cs], z_sb_r[:, cs],
                                              scale_t[:])
                  nc.gpsimd.tensor_scalar_add(zfin[:, cs], zfin[:, cs],
                                              bias2[:])
                  nc.sync.dma_start(zout_d[:, cs], zfin[:, cs])

          def emit_tail(cco, z_sb_r):
              mv = emit_tail_stats(cco)
              rstd = emit_tail_rstd(mv)
              emit_tail_apply(mv, rstd, z_sb_r)

          def emit_loads():
              # ---- DRAM loads, ordered for earliest compute start ----
              L = {}
              L['wt'] = [ldp.tile([128, CI], dt.float16, tag=f"wt{c}", name=f"wtT_r{c}") for c in range(2)]
              L['wp'] = [ldp.tile([128, CI], dt.float16, tag=f"wp{c}", name=f"wpT_r{c}") for c in range(2)]
              L['wzg'] = [ldp.tile([128, CI], dt.float16, tag=f"wz{c}", name=f"wzgT_r{c}") for c in range(2)]
              for c in range(2):
                  nc.sync.dma_start(L['wt'][c][:], wtT_d[c * 128:(c + 1) * 128, :])
                  nc.sync.dma_start(L['wp'][c][:], wpT_d[c * 128:(c + 1) * 128, :])
                  nc.sync.dma_start(L['wzg'][c][:], wzgT_d[c * 128:(c + 1) * 128, :])
              L['xo'] = [ldp.tile([128, NI], dt.float16, tag=f"xo{c}", name=f"xo_r{c}") for c in range(2)]
              for c in range(2):
                  nc.sync.dma_start(L['xo'][c][:, 0:512], xob[c * 128:(c + 1) * 128, 0:512])
              L['xt'] = [ldp.tile([128, NJ], dt.float16, tag=f"xt{c}", name=f"xt_r{c}") for c in range(2)]
              for jc in range(NJ // 1024):
                  cs = slice(jc * 1024, (jc + 1) * 1024)
                  for c in range(2):
                      nc.sync.dma_start(L['xt'][c][:, cs], xtb[c * 128:(c + 1) * 128, cs])
              for c in range(2):
                  nc.sync.dma_start(L['xo'][c][:, 512:NI], xob[c * 128:(c + 1) * 128, 512:NI])
              return L

          loads_cur = emit_loads()
          pre = {}

          for _rep in range(unroll):
              L = loads_cur
              wtT_r, wpT_r, wzgT_r = L['wt'], L['wp'], L['wzg']
              xo_r, xt_r = L['xo'], L['xt']

              # ---- projections (skipped when pre-projected last rep) ----
              if 'phi' in pre:
                  phi = pre.pop('phi')
              else:
                  phi = dbp.tile([128, NI], dt.float16, tag="phi")
                  ps = Sp.tile([128, 512], dt.float32, tag="S", name="ps_phi0")
                  for c in range(2):
                      nc.tensor.matmul(ps[:], wpT_r[c][:], xo_r[c][:, 0:512],
                                       start=(c == 0), stop=(c == 1))
                  nc.vector.tensor_scalar_add(phi[:, 0:512], ps[:], cst[:, 0:1])

              if 'theta' in pre:
                  theta = pre.pop('theta')
              else:
                  theta = dbp.tile([128, NJ], dt.float16, tag="theta")
                  for jc in range(NJ // 512):
                      ps = Sp.tile([128, 512], dt.float32, tag="S", name="ps_th")
                      for c in range(2):
                          nc.tensor.matmul(ps[:], wtT_r[c][:],
                                           xt_r[c][:, jc * 512:(jc + 1) * 512],
                                           start=(c == 0), stop=(c == 1))
                      # theta bias is constant per softmax row -> cancels
                      nc.vector.tensor_copy(theta[:, jc * 512:(jc + 1) * 512], ps[:])

              z_sb = dbp.tile([128, NI], dt.float32, tag="z_sb")
              gzT = dbp.tile([128, NJ], dt.bfloat16, tag="gzT")

              e_chunks = {}

              def emit_scores_group(k, g0, gn, interleave=None):
                  pk = slice(k * IC, (k + 1) * IC)
                  e3 = e_chunks[k][:].rearrange("p (i j) -> p j i", j=JBS)
                  S_ps = Sp.tile([128, 1024], dt.float32, tag="S", name="S_ps")
                  for jj in range(gn):
                      jb = g0 + jj
                      nc.tensor.matmul(S_ps[:, jj * IC:(jj + 1) * IC],
                                       theta[:, jb * 128:(jb + 1) * 128],
                                       phi[:, pk], start=True, stop=True)
                      if interleave is not None:
                          interleave(jb)
                  nc.scalar.activation(
                      e3[:, g0:g0 + gn, :],
                      S_ps[:].rearrange("p (j i) -> p j i", i=IC)[:, 0:gn, :],
                      AF.Exp, bias=cst[:, 3:4])

              def emit_gzT_block(jb):
                  ps = gzp.tile([128, 128], dt.float32, tag="gz", name="ps_gz")
                  for c in range(2):
                      nc.tensor.matmul(ps[:], xt_r[c][:, jb * 128:(jb + 1) * 128],
                                       wzgT_r[c][:], start=(c == 0), stop=(c == 1))
                  nc.vector.tensor_copy(gzT[:, jb * 128:(jb + 1) * 128], ps[:])

              # hoisted chunk 0: scores interleaved with gzT projection +
              # remaining phi chunks
              def chunk0_phi_chunk(jb):
                  # phi chunks 1..3 (cols 512..2048)
                  ps = Sp.tile([128, 512], dt.float32, tag="S", name="ps_phi")
                  cs = slice((jb + 1) * 512, (jb + 2) * 512)
                  for c in range(2):
                      nc.tensor.matmul(ps[:], wpT_r[c][:], xo_r[c][:, cs],
                                       start=(c == 0), stop=(c == 1))
                  nc.vector.tensor_scalar_add(phi[:, cs], ps[:], cst[:, 0:1])

              def chunk0_interleave(jb):
                  emit_gzT_block(jb)
                  if jb < 3:
                      chunk0_phi_chunk(jb)

              e_chunks[0] = expp.tile([128, IC * JBS], dt.bfloat16,
                                      tag="e", name="e0")
              g0 = 0
              for g, gn in enumerate(JGROUPS):
                  emit_scores_group(0, g0, gn, interleave=chunk0_interleave)
                  g0 += gn

              stat = dbp.tile([128, 2], dt.float32, tag="stat")

              def make_zprime_interleave(km1):
                  # one z'(chunk km1) matmul per score matmul of chunk km1+1
                  e3m = e_chunks[km1][:].rearrange("p (i j) -> p j i", j=JBS)
                  zpart = zpp.tile([128, IC], dt.float32, tag="zp", name="zpart")
                  st = {'m': 0}

                  def cb(_jb_score):
                      m = st['m']
                      st['m'] += 1
                      nc.tensor.matmul(zpart[:], gzT[:, m * 128:(m + 1) * 128],
                                       e3m[:, m, :], start=(m == 0),
                                       stop=(m == JBS - 1))
                  return cb, zpart

              def emit_chunk_reduce(k, zpart):
                  # softmax sums over jb for chunk k (DVE packed-bf16 tree for
                  # jb<24, GPSIMD tree for jb>=24), normalize + BN stat partials
                  pk = slice(k * IC, (k + 1) * IC)
                  ei = e_chunks[k][:].rearrange("p (i j) -> p i j", j=JBS)
                  t12 = smp.tile([128, IC * 12], dt.bfloat16, tag="t12", bufs=1)
                  t12v = t12[:].rearrange("p (i j) -> p i j", j=12)
                  nc.vector.tensor_add(t12v[:], ei[:, :, 0:12], ei[:, :, 12:24])
                  t6 = smp.tile([128, IC * 6], dt.bfloat16, tag="t6", bufs=1)
                  t6v = t6[:].rearrange("p (i j) -> p i j", j=6)
                  nc.vector.tensor_add(t6v[:], t12v[:, :, 0:6], t12v[:, :, 6:12])
                  t3 = smp.tile([128, IC * 3], dt.bfloat16, tag="t3", bufs=1)
                  t3v = t3[:].rearrange("p (i j) -> p i j", j=3)
                  nc.vector.tensor_add(t3v[:], t6v[:, :, 0:3], t6v[:, :, 3:6])
                  tx = smp.tile([128, IC], dt.float32, tag="tx", bufs=1)
                  nc.vector.tensor_add(tx[:], t3v[:, :, 0], t3v[:, :, 1])
                  s_bD = smp.tile([128, IC], dt.float32, tag="sbD", bufs=1)
                  nc.vector.tensor_add(s_bD[:], tx[:], t3v[:, :, 2])

                  u4 = smp.tile([128, IC * 4], dt.bfloat16, tag="u4", bufs=1)
                  u4v = u4[:].rearrange("p (i j) -> p i j", j=4)
                  nc.gpsimd.tensor_add(u4v[:], ei[:, :, 24:28], ei[:, :, 28:32])
                  u2 = smp.tile([128, IC * 2], dt.bfloat16, tag="u2", bufs=1)
                  u2v = u2[:].rearrange("p (i j) -> p i j", j=2)
                  nc.gpsimd.tensor_add(u2v[:], u4v[:, :, 0:2], u4v[:, :, 2:4])
                  s_bP = smp.tile([128, IC], dt.float32, tag="sbP", bufs=1)
                  nc.gpsimd.tensor_add(s_bP[:], u2v[:, :, 0], u2v[:, :, 1])

                  s_part = smp.tile([128, IC], dt.bfloat16, tag="sp")
                  nc.vector.tensor_add(s_part[:], s_bD[:], s_bP[:])
                  rs = rsp.tile([128, IC], dt.float32, tag="rs")
                  nc.tensor.matmul(rs[:], ones_b[:], s_part[:], start=True,
                                   stop=True)
                  rrs = smp.tile([128, IC], dt.float32, tag="rrs")
                  nc.vector.reciprocal(rrs[:], rs[:])

                  nc.vector.tensor_mul(z_sb[:, pk], zpart[:], rrs[:])
                  s1c = smp.tile([128, 1], dt.float32, tag="s1c")
                  nc.vector.reduce_sum(s1c[:], z_sb[:, pk],
                                       axis=mybir.AxisListType.X)
                  sq = smp.tile([128, IC], dt.float32, tag="sq")
                  nc.vector.tensor_mul(sq[:], z_sb[:, pk], z_sb[:, pk])
                  s2c = smp.tile([128, 1], dt.float32, tag="s2c")
                  nc.vector.reduce_sum(s2c[:], sq[:], axis=mybir.AxisListType.X)
                  if k == 0:
                      nc.vector.tensor_copy(stat[:, 0:1], s1c[:])
                      nc.vector.tensor_copy(stat[:, 1:2], s2c[:])
                  else:
                      nc.vector.tensor_add(stat[:, 0:1], stat[:, 0:1], s1c[:])
                      nc.vector.tensor_add(stat[:, 1:2], stat[:, 1:2], s2c[:])

              # chunks 1..7: score groups interleave the previous chunk's z'
              # matmuls so ACT never starves and PE never stalls on S_ps.
              # ch6 additionally pre-projects the next rep's theta; ch7
              # straddles 24 of this rep's chunk-7 z' matmuls (each needs its
              # exp group done, which runs 2 groups ahead).
              prefetch = _rep + 1 < unroll
              loads_next = None
              theta_next = None
              zpart7 = None
              for k in range(1, NCH):
                  e_chunks[k] = expp.tile([128, IC * JBS], dt.bfloat16,
                                          tag="e", name="e")
                  cb, zpart_prev = make_zprime_interleave(k - 1)
                  if k == 6 and prefetch:
                      theta_next = dbp.tile([128, NJ], dt.float16,
                                            tag="theta", name="theta_next")
                      Ln = loads_next
                      st6 = {'m': 0, 'ps': None}
                      base_cb = cb

                      def cb(_jb, base_cb=base_cb, Ln=Ln, st6=st6,
                             theta_next=theta_next):
                          base_cb(_jb)
                          m = st6['m']
                          st6['m'] += 1
                          if m < 16:
                              return
                          tj, c = (m - 16) // 2, (m - 16) % 2
                          if c == 0:
                              st6['ps'] = Sp.tile([128, 512], dt.float32,
                                                  tag="S", name="ps_thn")
                          nc.tensor.matmul(
                              st6['ps'][:], Ln['wt'][c][:],
                              Ln['xt'][c][:, tj * 512:(tj + 1) * 512],
                              start=(c == 0), stop=(c == 1))
                          if c == 1:
                              nc.vector.tensor_copy(
                                  theta_next[:, tj * 512:(tj + 1) * 512],
                                  st6['ps'][:])
                  elif k == 7:
                      e3m7 = e_chunks[7][:].rearrange("p (i j) -> p j i", j=JBS)
                      zpart7 = zpp.tile([128, IC], dt.float32, tag="zp",
                                        name="zpart7")
                      st7 = {'m': 0}
                      base_cb = cb

                      def cb(_jb, base_cb=base_cb, st7=st7):
                          base_cb(_jb)
                          m = st7['m']
                          st7['m'] += 1
                          if m < 8:
                              return
                          jb7 = m - 8
                          nc.tensor.matmul(zpart7[:],
                                           gzT[:, jb7 * 128:(jb7 + 1) * 128],
                                           e3m7[:, jb7, :], start=(jb7 == 0),
                                           stop=False)
                  g0 = 0
                  for g, gn in enumerate(JGROUPS):
                      emit_scores_group(k, g0, gn, interleave=cb)
                      g0 += gn
                  emit_chunk_reduce(k - 1, zpart_prev)
                  # previous rep's post-collective tail, in three stages so no
                  # queue ever head-of-line-waits on a cross-engine chain: by
                  # k==3 the AllGather (issued ~3 chunks ago) has completed
                  if k == 3 and pending_tail[0] is not None:
                      tail_mv[0] = emit_tail_stats(pending_tail[0][0])
                  if k == 4:
                      if prefetch:
                          loads_next = emit_loads()
                      if pending_tail[0] is not None:
                          tail_rstd[0] = emit_tail_rstd(tail_mv[0])
                  if k == 5 and pending_tail[0] is not None:
                      emit_tail_apply(tail_mv[0], tail_rstd[0],
                                      pending_tail[0][1])
                      pending_tail[0] = None

              # chunk 7's remaining z' matmuls (jb 24..31)
              e3m = e_chunks[NCH - 1][:].rearrange("p (i j) -> p j i", j=JBS)
              for jb in range(24, JBS):
                  nc.tensor.matmul(zpart7[:], gzT[:, jb * 128:(jb + 1) * 128],
                                   e3m[:, jb, :], start=False,
                                   stop=(jb == JBS - 1))
              emit_chunk_reduce(NCH - 1, zpart7)

              # next rep's phi cols 0:512
              if prefetch:
                  phi_next = dbp.tile([128, NI], dt.float16, tag="phi",
                                      name="phi_next")
                  ps = Sp.tile([128, 512], dt.float32, tag="S", name="ps_phn")
                  for c in range(2):
                      nc.tensor.matmul(ps[:], loads_next['wp'][c][:],
                                       loads_next['xo'][c][:, 0:512],
                                       start=(c == 0), stop=(c == 1))
                  nc.vector.tensor_scalar_add(phi_next[:, 0:512], ps[:],
                                              cst[:, 0:1])
                  pre = {'theta': theta_next, 'phi': phi_next}
                  loads_cur = loads_next

              # ---- BN stats AllGather (issue only; tail is deferred to the
              # middle of the next rep so its waits never block the queues) ----
              cci = cc_in[_rep % 2]
              cco = cc_out[_rep % 2]
              nc.sync.dma_start(cci.ap()[:], stat[:])
              nc.gpsimd.collective_compute(
                  "AllGather", mybir.AluOpType.bypass,
                  replica_groups=[list(range(8))],
                  ins=[cci.ap().opt()], outs=[cco.ap().opt()])
              pending_tail[0] = (cco, z_sb)

          emit_tail(*pending_tail[0])
          pending_tail[0] = None

    nc.compile()

    return nc


def _prep_in_maps(inputs):
    f16 = np.float16
    xt_full = inputs['x_thisBranch'].reshape(B, CIN, NJ).astype(f16)
    xo_full = inputs['x_otherBranch'].reshape(B, CIN, NJ).astype(f16)
    wtT = np.ascontiguousarray(inputs['w_theta'].T.astype(f16))
    wpT = np.ascontiguousarray(inputs['w_phi'].T.astype(f16))
    w_zg = (inputs['w_z'].astype(np.float64) @ inputs['w_g'].astype(np.float64))
    wzgT = np.ascontiguousarray(w_zg.T.astype(f16))
    consts = np.zeros((CI, 8), np.float32)
    consts[:, 0] = inputs['b_phi']
    consts[:, 1] = inputs['bn_gamma']
    consts[:, 2] = inputs['bn_beta']
    consts[:, 3] = SHIFT
    consts[:, 4] = BN_EPS
    consts[:, 5] = 1.0 / NTOT
    in_maps = []
    for c in range(8):
        b, h = c // 2, c % 2
        in_maps.append({
            "xtb": np.ascontiguousarray(xt_full[b]),
            "xob": np.ascontiguousarray(xo_full[b][:, h * NI:(h + 1) * NI]),
            "wtT": wtT, "wpT": wpT, "wzgT": wzgT, "consts": consts,
        })
    return in_maps


def kernel(**inputs):
    from concourse.bass_utils import run_bass_kernel_spmd
    if "nc" not in _CACHE:
        _CACHE["nc"] = _build()
    nc = _CACHE["nc"]
    in_maps = _prep_in_maps(inputs)
    res = run_bass_kernel_spmd(nc, in_maps, list(range(8)))
    out = np.empty((B, CI, NJ), np.float32)
    for c in range(8):
        b, h = c // 2, c % 2
        out[b][:, h * NI:(h + 1) * NI] = res.results[c]["z"]
    return out.reshape(B, CI, H, W)


if __name__ == "__main__":
    inputs = np.load('/tmp/ref_inputs.npy', allow_pickle=True).item()
    ref = np.load('/tmp/ref_output.npy')
    got = kernel(**inputs)
    err = np.abs(got - ref)
    denom = np.abs(ref).max()
    print(f"abs max err: {err.max():.4e}  (ref absmax {denom:.3f})")
    print(f"Relative error: {err.max() / denom:.4e}")
